# revision 38
# baseline (speedup 1.0000x reference)
"""Trainium2 Bass kernel for nn_AttnFree_Block (AFT + neural adaptive pairwise bias).

Sharding: 8 cores over the (B=2, T=512) query-row grid -> 128 query rows/core.
Each core computes the full pairwise bias network for its 128 rows x 512 keys,
then the AFT attention + FFN for its rows. Output rows are gathered on host.

v1: software-pipelined bias-net loop. All consumers of e (gate hidden, channel
sums, stat projections) are refactored to read h via host-precomputed composite
weights (w2/gamma/beta folds), so the per-chunk cross-engine chain is just
h-mm -> silu -> 5 independent matmuls. Stats pack 16 chunks per PSUM tile.
fe rows are DMA broadcast-gathered instead of selector matmuls.
"""
import sys
sys.path.insert(0, '/opt/trn_rl_repo')

import math
import numpy as np

import concourse.bass as bass
import concourse.bacc as bacc
import concourse.mybir as mybir
import concourse.tile as tile
from concourse import bass_utils

F32 = mybir.dt.float32
F32R = mybir.dt.float32r
BF16 = mybir.dt.bfloat16

AF = mybir.ActivationFunctionType
ALU = mybir.AluOpType
AX = mybir.AxisListType

B, T, D, H, HID, FF = 2, 512, 128, 128, 64, 512
NCORES = 8
RPC = T * B // NCORES  # 128 query rows per core
FREQS = (1.0, 2.0, 4.0, 8.0)
EPS_LN = 1e-5
EPS_RMS = 1e-5
EPSX = 1e-4          # epsilon for the sin(eps*x)/eps ~= x identity-row trick
PI = math.pi

_CACHE = {}


def _build_program():
    nc = bacc.Bacc()

    def din(name, shape, dt=F32):
        return nc.dram_tensor(name, list(shape), dt, kind="ExternalInput")

    t_xT = din("xT", (D, T), F32R)
    t_xrows = din("xrows", (RPC, D))
    t_xrowsT = din("xrowsT", (D, RPC), F32R)
    t_cost = din("cost_r", (RPC, T))      # cost_mat rows for this core
    t_clx = din("clx", (2, RPC))
    t_cly = din("cly", (2, RPC))
    t_crx = din("crx", (2, T))
    t_cry = din("cry", (2, T))

    t_w1big = din("w1big", (96, 384), F32R)
    t_sinscale = din("sinscale3", (96, 1))
    t_sinbias = din("sinbias3", (96, 1))
    t_b1s = din("b1s", (128, 1))
    t_We = din("We", (128, 128), F32R)        # gamma-scaled blockdiag w2
    t_Wss2 = din("Wss2", (128, 256), F32R)    # even|odd pair sum-pattern
    t_Wgg2 = din("Wgg2", (128, 256), F32R)    # even|odd gate-hidden pattern
    t_gb1s = din("gb1s", (128, 1))
    t_bs128 = din("bs128", (128, 1))          # beta0+beta1 (sq-ss bias)
    t_WstE = din("WstE", (128, 143), F32R)    # sliding stat weights from h
    t_WstQ = din("WstQ", (128, 143), F32R)    # sliding stat weights from e^2
    t_WstS = din("WstS", (128, 142), F32R)    # sliding qss weights from ss^2
    t_WstG = din("WstG", (128, 142), F32R)    # sliding d weights from silu(gg)
    t_ident = din("ident", (128, 128))

    t_wq = din("wq", (D, H), F32R)
    t_wk = din("wk", (D, H), F32R)
    t_wv = din("wv", (D, H), F32R)
    t_wo = din("wo", (H, D), F32R)
    t_bq = din("bq", (H, 1))
    t_bvb = din("bvb", (128, H))
    t_bob = din("bob", (RPC, D))
    t_rms1b = din("rms1b", (RPC, D))
    t_rms2b = din("rms2b", (RPC, D))
    t_ffw1 = din("ffw1", (D, FF), F32R)
    t_ffb1s = din("ffb1s", (128, 4))
    t_ffw2 = din("ffw2", (D, FF), F32R)
    t_ffb2b = din("ffb2b", (RPC, D))

    t_angle_dram = nc.dram_tensor("angle_scratch", [RPC, T], F32, kind="Internal")
    t_out = nc.dram_tensor("out", [RPC, D], F32, kind="ExternalOutput")
    return nc, locals()


def _emit(nc, tt, consts):
    gb2d = consts["gb2d"]
    gps64 = consts["gps64"]
    NCH = RPC  # 128 chunks (query rows) per core

    with tile.TileContext(nc) as tc:
        import contextlib
        with contextlib.ExitStack() as ctx:
            singles = ctx.enter_context(tc.tile_pool(name="singles", bufs=1))
            sb = ctx.enter_context(tc.tile_pool(name="sb", bufs=1))

            def load1(t, shape, pool=None):
                p = pool or singles
                s = p.tile(list(shape), t.dtype, tag=t.name + "_sb", name=t.name + "_sb")
                nc.sync.dma_start(out=s[:, :], in_=t.ap()[:, :])
                return s

            # ---- load weights/statics (phase A + D-prep inputs first) ----
            clx = load1(tt["t_clx"], (2, RPC))
            cly = load1(tt["t_cly"], (2, RPC))
            crx = load1(tt["t_crx"], (2, T))
            cry = load1(tt["t_cry"], (2, T))
            xT = load1(tt["t_xT"], (D, T))
            xrowsT = load1(tt["t_xrowsT"], (D, RPC))
            wq = load1(tt["t_wq"], (D, H))
            wk = load1(tt["t_wk"], (D, H))
            wv = load1(tt["t_wv"], (D, H))
            ident = load1(tt["t_ident"], (128, 128))
            bq = load1(tt["t_bq"], (H, 1))
            bvb = load1(tt["t_bvb"], (128, H))
            w1big = load1(tt["t_w1big"], (96, 384))
            sinscale = load1(tt["t_sinscale"], (96, 1))
            sinbias = load1(tt["t_sinbias"], (96, 1))
            b1s = load1(tt["t_b1s"], (128, 1))
            We = load1(tt["t_We"], (128, 128))
            Wss2 = load1(tt["t_Wss2"], (128, 256))
            Wgg2 = load1(tt["t_Wgg2"], (128, 256))
            gb1s = load1(tt["t_gb1s"], (128, 1))
            WstE = load1(tt["t_WstE"], (128, 143))
            WstQ = load1(tt["t_WstQ"], (128, 143))
            WstS = load1(tt["t_WstS"], (128, 142))
            WstG = load1(tt["t_WstG"], (128, 142))
            bs128 = load1(tt["t_bs128"], (128, 1))

            def load_tail_weights():
                return (load1(tt["t_xrows"], (RPC, D)),
                        load1(tt["t_wo"], (H, D)),
                        load1(tt["t_bob"], (RPC, D)),
                        load1(tt["t_rms1b"], (RPC, D)),
                        load1(tt["t_rms2b"], (RPC, D)),
                        load1(tt["t_ffw1"], (D, FF)),
                        load1(tt["t_ffb1s"], (128, 4)),
                        load1(tt["t_ffw2"], (D, FF)),
                        load1(tt["t_ffb2b"], (RPC, D)))

            c_epsln = singles.tile([128, 1], F32, tag="c_epsln")
            nc.vector.memset(c_epsln[:, :], EPS_LN)
            c_epsrms = singles.tile([128, 1], F32, tag="c_epsrms")
            nc.vector.memset(c_epsrms[:, :], EPS_RMS)
            c_gb2d = singles.tile([128, 1], F32, tag="c_gb2d")
            nc.vector.memset(c_gb2d[:, :], gb2d)

            # ================= Phase A: angle matrix =================
            with tc.tile_pool(name="pA", bufs=2, space="PSUM") as pA:
                dx_ps = pA.tile([RPC, T], F32, tag="dxy")
                nc.tensor.matmul(dx_ps[:, :], clx[:, :], crx[:, :], start=True, stop=True)
                dy_ps = pA.tile([RPC, T], F32, tag="dxy")
                nc.tensor.matmul(dy_ps[:, :], cly[:, :], cry[:, :], start=True, stop=True)
                sgn = sb.tile([RPC, T], F32, tag="sgn")
                nc.scalar.sign(sgn[:, :], dx_ps[:, :])
                dxa = sb.tile([RPC, T], F32, tag="dxa")
                nc.vector.tensor_mul(dxa[:, :], sgn[:, :], dx_ps[:, :])
                cmpneg = sb.tile([RPC, T], F32, tag="cmpneg")
                nc.vector.tensor_scalar(cmpneg[:, :], sgn[:, :], 0.0, None, op0=ALU.is_lt)
                sdy = sb.tile([RPC, T], F32, tag="sdy")
                nc.vector.tensor_scalar(sdy[:, :], dy_ps[:, :], 0.0, None, op0=ALU.is_ge)
                nc.vector.tensor_scalar(sdy[:, :], sdy[:, :], 2.0, -1.0, op0=ALU.mult, op1=ALU.add)
                dya = sb.tile([RPC, T], F32, tag="dya")
                nc.vector.tensor_mul(dya[:, :], sdy[:, :], dy_ps[:, :])
                mx = sb.tile([RPC, T], F32, tag="mx")
                nc.vector.tensor_tensor(out=mx[:, :], in0=dxa[:, :], in1=dya[:, :], op=ALU.max)
                nc.vector.tensor_scalar_max(mx[:, :], mx[:, :], 1e-30)
                mn = sb.tile([RPC, T], F32, tag="mn")
                nc.vector.tensor_tensor(out=mn[:, :], in0=dxa[:, :], in1=dya[:, :], op=ALU.min)
                nc.vector.reciprocal(mx[:, :], mx[:, :])
                rt = sb.tile([RPC, T], F32, tag="rt")
                nc.vector.tensor_mul(rt[:, :], mn[:, :], mx[:, :])
                at = sb.tile([RPC, T], F32, tag="at")
                nc.scalar.activation(at[:, :], rt[:, :], AF.Arctan)
                swap = sb.tile([RPC, T], F32, tag="swap")
                nc.vector.tensor_tensor(out=swap[:, :], in0=dya[:, :], in1=dxa[:, :], op=ALU.is_gt)
                v1 = sb.tile([RPC, T], F32, tag="v1")
                nc.vector.tensor_scalar(v1[:, :], at[:, :], -2.0, PI / 2, op0=ALU.mult, op1=ALU.add)
                nc.vector.tensor_mul(v1[:, :], v1[:, :], swap[:, :])
                base = sb.tile([RPC, T], F32, tag="base")
                nc.vector.tensor_add(base[:, :], at[:, :], v1[:, :])
                v2 = sb.tile([RPC, T], F32, tag="v2")
                nc.vector.tensor_scalar(v2[:, :], base[:, :], -2.0, PI, op0=ALU.mult, op1=ALU.add)
                nc.vector.tensor_mul(v2[:, :], v2[:, :], cmpneg[:, :])
                nc.vector.tensor_add(base[:, :], base[:, :], v2[:, :])
                angle = sb.tile([RPC, T], F32, tag="angle")
                nc.vector.tensor_mul(angle[:, :], sdy[:, :], base[:, :])
                nc.sync.dma_start(out=tt["t_angle_dram"].ap()[:, :], in_=angle[:, :])

            # ============ Phase D-prep: Q_sig, K-softmax, eK, eKT, ekv ============
            qs_sb = singles.tile([H, RPC], F32, tag="qs_sb")
            eKT = singles.tile([128, T], F32R, tag="eKT")
            ekv = singles.tile([128, T], F32R, tag="ekv")
            with tc.tile_pool(name="dprep", bufs=1) as dprep, \
                 tc.tile_pool(name="psW", bufs=2, space="PSUM") as psW, \
                 tc.tile_pool(name="psK", bufs=1, space="PSUM") as psK:
                q_ps = psW.tile([H, RPC], F32, tag="q_ps")
                nc.tensor.matmul(q_ps[:, :], wq[:, :], xrowsT[:, :], start=True, stop=True)
                nc.scalar.activation(qs_sb[:, :], q_ps[:, :], AF.Sigmoid, bias=bq[:, :])

                kl_ps = psK.tile([H, T], F32, tag="kl_ps")
                nc.tensor.matmul(kl_ps[:, :], wk[:, :], xT[:, :], start=True, stop=True)
                kmax = dprep.tile([H, 1], F32, tag="kmax")
                nc.vector.reduce_max(kmax[:, :], kl_ps[:, :], axis=AX.X, negate=True)
                kex = dprep.tile([H, T], F32, tag="kex")
                nc.scalar.activation(kex[:, :], kl_ps[:, :], AF.Exp, bias=kmax[:, :])
                ksum = dprep.tile([H, 1], F32, tag="ksum")
                nc.vector.reduce_sum(ksum[:, :], kex[:, :], axis=AX.X)
                nc.vector.reciprocal(ksum[:, :], ksum[:, :])
                eK = dprep.tile([H, T], F32, tag="eK")
                nc.scalar.activation(eK[:, :], kex[:, :], AF.Exp, scale=ksum[:, :])
                for tb in range(4):
                    tp = psW.tile([128, 128], F32, tag="scrW")
                    nc.tensor.transpose(tp[:, :], eK[:, 128 * tb:128 * tb + 128], ident[:, :])
                    nc.vector.tensor_copy(out=eKT[:, 128 * tb:128 * tb + 128], in_=tp[:, :])
                for tb in range(4):
                    v_ps = psW.tile([128, H], F32, tag="scrW")
                    nc.tensor.matmul(v_ps[:, :], xT[:, 128 * tb:128 * tb + 128], wv[:, :],
                                     start=True, stop=True)
                    vb = dprep.tile([128, H], F32R, tag="vb")
                    nc.vector.tensor_add(vb[:, :], v_ps[:, :], bvb[:, :])
                    nc.vector.tensor_mul(ekv[:, 128 * tb:128 * tb + 128], vb[:, :],
                                         eKT[:, 128 * tb:128 * tb + 128])

            # ================= Phase B: pipelined bias-net main loop =================
            NG = (NCH + 2) // 3  # fe groups of 3 chunks (base partitions 0/32/64)
            Sb = [singles.tile([128, T], F32, tag=f"Sblk{b}", name=f"Sblk{b}")
                  for b in range(NCH // 16)]

            with tc.tile_pool(name="feRaw", bufs=2) as feRaw, \
                 tc.tile_pool(name="feSb", bufs=2) as feSb, \
                 tc.tile_pool(name="hSb", bufs=3) as hSb, \
                 tc.tile_pool(name="sqSb", bufs=2) as sqSb, \
                 tc.tile_pool(name="ssSb", bufs=2) as ssSb, \
                 tc.tile_pool(name="ggSb", bufs=2) as ggSb, \
                 tc.tile_pool(name="psHE", bufs=3, space="PSUM") as psHE, \
                 tc.tile_pool(name="psSG", bufs=3, space="PSUM") as psSG, \
                 tc.tile_pool(name="psST", bufs=2, space="PSUM") as psST:

                st = {"fe_raw": {}, "fe_sb": {}, "h_ps": {}, "h_sb": {}, "e_ps": {},
                      "sq_sb": {}, "ss_ps": {}, "gg_ps": {}, "ss_sb": {}, "gg_sb": {},
                      "stats": {}}
                prologue = True

                def dma_fe(g):
                    n = min(3, NCH - 3 * g)
                    fr = feRaw.tile([96, T], F32, tag="fe_raw")
                    st["fe_raw"][g] = (fr, n)
                    if g < 2:
                        nc.vector.memset(fr[:, :], 0.0)
                    for c in range(n):
                        i = 3 * g + c
                        src_c = bass.AP(tensor=tt["t_cost"], offset=i * T, ap=[[0, 9], [1, T]])
                        nc.sync.dma_start(out=fr[32 * c:32 * c + 9, :], in_=src_c)
                        src_a = bass.AP(tensor=tt["t_angle_dram"], offset=i * T, ap=[[0, 9], [1, T]])
                        nc.sync.dma_start(out=fr[32 * c + 9:32 * c + 18, :], in_=src_a)

                def sin_g(g):
                    fr, n = st["fe_raw"][g]
                    fs = feSb.tile([96, T], F32R, tag="fe_sb")
                    st["fe_sb"][g] = fs
                    nc.scalar.activation(fs[:, :], fr[:, :], AF.Sin,
                                         scale=sinscale[:, :], bias=sinbias[:, :])

                def h_mm(c):
                    g, c3 = c // 3, c % 3
                    fs = st["fe_sb"][g]
                    hp = psHE.tile([128, T], F32, tag="he_ps")
                    st["h_ps"][c] = hp
                    nc.tensor.matmul(hp[:, :], w1big[:, 128 * c3:128 * c3 + 128],
                                     fs[0:96, :], start=True, stop=True)

                def silu_h(c):
                    hp = st["h_ps"].pop(c)
                    hs = hSb.tile([128, T], F32R, tag="h_sb")
                    st["h_sb"][c] = hs
                    nc.scalar.activation(hs[:, :], hp[:, :], AF.Silu, bias=b1s[:, :])

                for t in range(-2, NCH + 3):
                    if prologue:
                        dma_fe(0)
                        dma_fe(1)
                        sin_g(0)
                        prologue = False
                    # vector: square of pair sums (reads ss PSUM once), frees ss_ps
                    if t % 2 == 0 and 0 <= (t - 2) // 2 < NCH // 2:
                        p = (t - 2) // 2
                        ssp = st["ss_ps"].pop(p)
                        tmp = ssSb.tile([128, T], F32, tag="ss_tmp")
                        nc.vector.tensor_scalar(tmp[:, :], ssp[:, :], bs128[:, :],
                                                None, op0=ALU.add)
                        sss = ssSb.tile([128, T], F32R, tag="ss_sb")
                        st["ss_sb"][p] = sss
                        nc.vector.tensor_mul(sss[:, :], tmp[:, :], tmp[:, :])
                    # scalar: silu of pair gate hidden (reads PSUM), frees gg_ps
                    if t % 2 == 0 and 0 <= (t - 2) // 2 < NCH // 2:
                        p = (t - 2) // 2
                        ggp = st["gg_ps"].pop(p)
                        ggs = ggSb.tile([128, T], F32R, tag="gg_sb")
                        st["gg_sb"][p] = ggs
                        nc.scalar.activation(ggs[:, :], ggp[:, :], AF.Silu, bias=gb1s[:, :])
                    # tensor: h two slots ahead; scalar: its silu right after
                    if 0 <= t + 2 < NCH:
                        h_mm(t + 2)
                        silu_h(t + 2)
                    # scalar: sin for group (t+3)//3 (3-slot lead)
                    if (t + 3) % 3 == 0 and 0 < (t + 3) // 3 < NG:
                        sin_g((t + 3) // 3)
                    # dma: fe gather for group t//3+2 (5-slot lead)
                    if t % 3 == 0 and 1 < t // 3 + 2 < NG:
                        dma_fe(t // 3 + 2)

                    if 0 <= t < NCH:
                        c = t
                        c16, hp2 = c % 16, c % 2
                        hs = st["h_sb"][c]
                        # e-hat matmul
                        ep = psHE.tile([128, T], F32, tag="he_ps")
                        st["e_ps"][c] = ep
                        nc.tensor.matmul(ep[:, :], We[:, :], hs[:, :], start=True, stop=True)
                        if hp2 == 0:
                            ssp = psSG.tile([128, T], F32, tag="sg_ps")
                            ggp = psSG.tile([128, T], F32, tag="sg_ps")
                            st["ss_ps"][c // 2] = ssp
                            st["gg_ps"][c // 2] = ggp
                        ssp = st["ss_ps"][c // 2]
                        nc.tensor.matmul(ssp[:, :], Wss2[:, 128 * hp2:128 * hp2 + 128],
                                         hs[:, :], start=(hp2 == 0), stop=(hp2 == 1))
                        # linear stats from h
                        if c16 == 0:
                            stp = psST.tile([128, T], F32, tag="st_ps")
                            st["stats"][c // 16] = stp
                        stp = st["stats"][c // 16]
                        nc.tensor.matmul(stp[:, :], WstE[:, 15 - c16:143 - c16],
                                         hs[:, :], start=(c16 == 0), stop=False)
                        # square of e-hat: alternate engines to balance load.
                        # odd chunks: scalar straight from PSUM; even chunks:
                        # vector copy + gpsimd square (gpsimd is otherwise idle).
                        sqs = sqSb.tile([128, T], F32R, tag="sq_sb")
                        st["sq_sb"][c] = sqs
                        if c % 2 == 1:
                            nc.scalar.activation(sqs[:, :], ep[:, :], AF.Square)
                        else:
                            elin = sqSb.tile([128, T], F32R, tag="e_lin")
                            nc.vector.tensor_copy(out=elin[:, :], in_=ep[:, :])
                            nc.gpsimd.tensor_mul(sqs[:, :], elin[:, :], elin[:, :])

                    # tensor: quadratic stats of previous chunk
                    if 0 <= t - 1 < NCH:
                        c = t - 1
                        c16 = c % 16
                        sqs = st["sq_sb"].pop(c)
                        stp = st["stats"][c // 16]
                        nc.tensor.matmul(stp[:, :], WstQ[:, 15 - c16:143 - c16],
                                         sqs[:, :], start=False, stop=False)
                        st["e_ps"].pop(c, None)

                    # tensor: pair stats (qss, d)
                    if t % 2 == 1 and 0 <= (t - 3) // 2 < NCH // 2:
                        p = (t - 3) // 2
                        p8 = p % 8
                        sss = st["ss_sb"].pop(p)
                        ggs = st["gg_sb"].pop(p)
                        stp = st["stats"][p // 8]
                        nc.tensor.matmul(stp[:, :], WstS[:, 14 - 2 * p8:142 - 2 * p8],
                                         sss[:, :], start=False, stop=False)
                        nc.tensor.matmul(stp[:, :], WstG[:, 14 - 2 * p8:142 - 2 * p8],
                                         ggs[:, :], start=False, stop=(p8 == 7))

                    # tensor: gg accumulation last (bufs=3 rotation hazard dodge)
                    if 0 <= t < NCH:
                        c = t
                        hp2 = c % 2
                        hs = st["h_sb"][c]
                        ggp = st["gg_ps"][c // 2]
                        nc.tensor.matmul(ggp[:, :], Wgg2[:, 128 * hp2:128 * hp2 + 128],
                                         hs[:, :], start=(hp2 == 0), stop=(hp2 == 1))
                        st["h_sb"].pop(c, None)

                    # vector: completed stats block PSUM -> SBUF
                    if t >= 18 and (t - 18) % 16 == 0 and (t - 18) // 16 < NCH // 16:
                        b = (t - 18) // 16
                        stp = st["stats"].pop(b)
                        nc.vector.tensor_copy(out=Sb[b][:, :], in_=stp[:, :])

            (xrows, wo, bob, rms1b, rms2b, ffw1, ffb1s, ffw2, ffb2b) = load_tail_weights()

            # ================= Phase B2: transpose stats to key-major =================
            T_coll = singles.tile([128, 4096], F32, tag="Tcoll")
            TvS = T_coll[:, :].rearrange("p (s jb i) -> p jb s i", s=8, jb=4)
            with tc.tile_pool(name="psTr", bufs=2, space="PSUM") as psTr:
                for b in range(NCH // 16):
                    trp = psTr.tile([128, 512], F32, tag="trp")
                    for jb in range(4):
                        nc.tensor.transpose(trp[:, 128 * jb:128 * jb + 128],
                                            Sb[b][:, 128 * jb:128 * jb + 128], ident[:, :])
                    nc.vector.tensor_copy(
                        out=TvS[:, :, :, 16 * b:16 * b + 16],
                        in_=trp[:, :].rearrange("p (jb s i) -> p jb s i", jb=4, s=8))

            # ================= Phase C: assemble adaptive bias =================
            ab_sb = singles.tile([128, T], F32, tag="ab_sb")
            with tc.tile_pool(name="ph2", bufs=1) as ph2, \
                 tc.tile_pool(name="psC", bufs=2, space="PSUM") as psC:
                S = [T_coll[:, 512 * k:512 * k + 512] for k in range(8)]

                def dt(tag):
                    return ph2.tile([128, T], F32, tag=tag, name=tag)

                def cadd(ap, const):
                    if const != 0.0:
                        nc.vector.tensor_scalar(ap, ap, const, None, op0=ALU.add)

                g0t = dt("g0t")
                nc.scalar.activation(g0t[:, :], S[7], AF.Sigmoid, bias=c_gb2d[:, :])

                # Stat weights are pre-scaled host-side (a-slots 1/64, q-slots
                # 1/64) so the whole chain is tensor-tensor ops, splittable
                # across engines. s0 = mean, s2 = E[f^2], P = 2*q01/64.
                za = dt("za"); s0 = dt("s0"); zb = dt("zb"); s1 = dt("s1")
                t1 = dt("t1"); qp = dt("qp"); ss5 = dt("ss5"); zq2 = dt("zq2")
                spq = dt("spq"); zv = dt("zv"); v1 = dt("v1"); v2 = dt("v2")
                v3 = dt("v3"); s2 = dt("s2"); m2 = dt("m2")
                var = dt("var")
                rstd = dt("rstd")
                num1 = dt("num1"); abT = dt("abT")
                SPL = 320

                def half(eng, a, b):
                    cut = lambda ap: ap[:, a:b]
                    Sh = [S[k][:, a:b] for k in range(8)]

                    def TT(o, i0, i1, op):
                        eng.tensor_tensor(out=o, in0=i0, in1=i1, op=op)

                    def cadd(ap, const):
                        if const != 0.0:
                            nc.vector.tensor_scalar(ap, ap, const, None, op0=ALU.add)

                    TT(cut(za[:, :]), Sh[0], Sh[1], ALU.subtract)
                    cadd(cut(za[:, :]), (consts["ca0"] - consts["ca1"]) / 64.0)
                    TT(cut(s0[:, :]), cut(g0t[:, :]), cut(za[:, :]), ALU.mult)
                    TT(cut(s0[:, :]), cut(s0[:, :]), Sh[1], ALU.add)
                    cadd(cut(s0[:, :]), consts["ca1"] / 64.0)
                    TT(cut(zb[:, :]), Sh[2], Sh[3], ALU.subtract)
                    cadd(cut(zb[:, :]), consts["cb0"] - consts["cb1"])
                    TT(cut(s1[:, :]), cut(g0t[:, :]), cut(zb[:, :]), ALU.mult)
                    TT(cut(s1[:, :]), cut(s1[:, :]), Sh[3], ALU.add)
                    cadd(cut(s1[:, :]), consts["cb1"])
                    # P = 2*q01/64 = S6 - S4 - S5 (all 1/64-scaled)
                    TT(cut(t1[:, :]), Sh[6], Sh[4], ALU.subtract)
                    TT(cut(qp[:, :]), cut(t1[:, :]), Sh[5], ALU.subtract)
                    cadd(cut(qp[:, :]), (-consts["cq00"] - consts["cq11"]) / 64.0)
                    TT(cut(ss5[:, :]), Sh[5], Sh[5], ALU.add)
                    TT(cut(zq2[:, :]), cut(qp[:, :]), cut(ss5[:, :]), ALU.subtract)
                    cadd(cut(zq2[:, :]), -2.0 * consts["cq11"] / 64.0)
                    TT(cut(spq[:, :]), Sh[4], Sh[5], ALU.add)
                    cadd(cut(spq[:, :]), (consts["cq00"] + consts["cq11"]) / 64.0)
                    TT(cut(zv[:, :]), cut(spq[:, :]), cut(qp[:, :]), ALU.subtract)
                    TT(cut(v1[:, :]), cut(g0t[:, :]), cut(zv[:, :]), ALU.mult)
                    TT(cut(v2[:, :]), cut(zq2[:, :]), cut(v1[:, :]), ALU.add)
                    TT(cut(v3[:, :]), cut(g0t[:, :]), cut(v2[:, :]), ALU.mult)
                    TT(cut(s2[:, :]), cut(v3[:, :]), Sh[5], ALU.add)
                    cadd(cut(s2[:, :]), consts["cq11"] / 64.0)
                    TT(cut(m2[:, :]), cut(s0[:, :]), cut(s0[:, :]), ALU.mult)
                    TT(cut(var[:, :]), cut(s2[:, :]), cut(m2[:, :]), ALU.subtract)

                half(nc.vector, 0, SPL)
                half(nc.gpsimd, SPL, T)
                sdv = dt("sdv")
                nc.scalar.activation(sdv[:, :], var[:, :], AF.Sqrt, bias=c_epsln[:, :])
                nc.vector.reciprocal(rstd[:, :], sdv[:, :])
                gpsum = consts["gps64"] * 64.0
                nc.vector.scalar_tensor_tensor(num1[:, :], s0[:, :], -gpsum, s1[:, :],
                                               op0=ALU.mult, op1=ALU.add)
                nc.vector.tensor_mul(abT[:, 0:SPL], num1[:, 0:SPL], rstd[:, 0:SPL])
                nc.gpsimd.tensor_mul(abT[:, SPL:T], num1[:, SPL:T], rstd[:, SPL:T])
                for jb in range(4):
                    tp = psC.tile([128, 128], F32, tag="tpC")
                    nc.tensor.transpose(tp[:, :], abT[:, 128 * jb:128 * jb + 128], ident[:, :])
                    nc.vector.tensor_copy(out=ab_sb[:, 128 * jb:128 * jb + 128], in_=tp[:, :])

            # ================= Phase D: AFT attention + FFN =================
            with tc.tile_pool(name="phD", bufs=1) as phD, \
                 tc.tile_pool(name="psD", bufs=3, space="PSUM") as psD, \
                 tc.tile_pool(name="psD2", bufs=1, space="PSUM") as psD2:
                rmax = phD.tile([128, 1], F32, tag="rmax")
                nc.vector.reduce_max(rmax[:, :], ab_sb[:, :], axis=AX.X, negate=True)
                ex = phD.tile([128, T], F32, tag="ex")
                nc.scalar.activation(ex[:, :], ab_sb[:, :], AF.Exp, bias=rmax[:, :])
                rsum = phD.tile([128, 1], F32, tag="rsum")
                nc.vector.reduce_sum(rsum[:, :], ex[:, :], axis=AX.X)
                nc.vector.reciprocal(rsum[:, :], rsum[:, :])
                A_sb = phD.tile([128, T], F32, tag="A_sb")
                nc.scalar.activation(A_sb[:, :], ex[:, :], AF.Exp, scale=rsum[:, :])
                AT_sb = phD.tile([128, T], F32R, tag="AT_sb")
                for jb in range(4):
                    tp = psD.tile([128, 128], F32, tag="scrD")
                    nc.tensor.transpose(tp[:, :], A_sb[:, 128 * jb:128 * jb + 128], ident[:, :])
                    nc.vector.tensor_copy(out=AT_sb[:, 128 * jb:128 * jb + 128], in_=tp[:, :])

                num_ps = psD2.tile([H, RPC], F32, tag="num_ps")
                den_ps = psD2.tile([H, RPC], F32, tag="den_ps")
                for jb in range(4):
                    nc.tensor.matmul(num_ps[:, :], ekv[:, 128 * jb:128 * jb + 128],
                                     AT_sb[:, 128 * jb:128 * jb + 128],
                                     start=(jb == 0), stop=(jb == 3))
                for jb in range(4):
                    nc.tensor.matmul(den_ps[:, :], eKT[:, 128 * jb:128 * jb + 128],
                                     AT_sb[:, 128 * jb:128 * jb + 128],
                                     start=(jb == 0), stop=(jb == 3))
                rden = phD.tile([H, RPC], F32, tag="rden")
                nc.vector.reciprocal(rden[:, :], den_ps[:, :])
                attT = phD.tile([H, RPC], F32R, tag="attT")
                nc.vector.tensor_mul(attT[:, :], num_ps[:, :], rden[:, :])
                nc.vector.tensor_mul(attT[:, :], attT[:, :], qs_sb[:, :])

                att_ps = psD2.tile([RPC, D], F32, tag="att_ps")
                nc.tensor.matmul(att_ps[:, :], attT[:, :], wo[:, :], start=True, stop=True)
                r1 = phD.tile([RPC, D], F32, tag="r1")
                nc.vector.tensor_add(r1[:, :], att_ps[:, :], xrows[:, :])
                nc.vector.tensor_add(r1[:, :], r1[:, :], bob[:, :])

                sq1 = phD.tile([RPC, D], F32, tag="sq1")
                nc.gpsimd.tensor_mul(sq1[:, :], r1[:, :], r1[:, :])
                ms = phD.tile([RPC, 1], F32, tag="ms")
                nc.vector.reduce_sum(ms[:, :], sq1[:, :], axis=AX.X)
                nc.scalar.activation(ms[:, :], ms[:, :], AF.Sqrt, scale=1.0 / D, bias=c_epsrms[0:RPC, :])
                nc.vector.reciprocal(ms[:, :], ms[:, :])
                h1 = phD.tile([RPC, D], F32, tag="h1")
                nc.vector.tensor_scalar_mul(h1[:, :], r1[:, :], ms[:, :])
                nc.vector.tensor_mul(h1[:, :], h1[:, :], rms1b[:, :])

                h1T_ps = psD.tile([D, RPC], F32, tag="scrD")
                nc.tensor.transpose(h1T_ps[:, :], h1[:, :], ident[:, :])
                h1T = phD.tile([D, RPC], F32R, tag="h1T")
                nc.vector.tensor_copy(out=h1T[:, :], in_=h1T_ps[:, :])

                relu_sb = phD.tile([128, FF], F32R, tag="relu_sb")
                for fb in range(4):
                    f_ps = psD.tile([128, RPC], F32, tag="scrD")
                    nc.tensor.matmul(f_ps[:, :], ffw1[:, 128 * fb:128 * fb + 128], h1T[:, :],
                                     start=True, stop=True)
                    nc.scalar.activation(relu_sb[:, 128 * fb:128 * fb + 128], f_ps[:, :],
                                         AF.Relu, bias=ffb1s[:, fb:fb + 1])
                o2_ps = psD2.tile([RPC, D], F32, tag="o2_ps")
                for fb in range(4):
                    nc.tensor.matmul(o2_ps[:, :], relu_sb[:, 128 * fb:128 * fb + 128],
                                     ffw2[:, 128 * fb:128 * fb + 128],
                                     start=(fb == 0), stop=(fb == 3))
                r2 = phD.tile([RPC, D], F32, tag="r2")
                nc.vector.tensor_add(r2[:, :], o2_ps[:, :], h1[:, :])
                nc.vector.tensor_add(r2[:, :], r2[:, :], ffb2b[:, :])

                sq2 = phD.tile([RPC, D], F32, tag="sq2")
                nc.gpsimd.tensor_mul(sq2[:, :], r2[:, :], r2[:, :])
                ms2 = phD.tile([RPC, 1], F32, tag="ms2")
                nc.vector.reduce_sum(ms2[:, :], sq2[:, :], axis=AX.X)
                nc.scalar.activation(ms2[:, :], ms2[:, :], AF.Sqrt, scale=1.0 / D, bias=c_epsrms[0:RPC, :])
                nc.vector.reciprocal(ms2[:, :], ms2[:, :])
                outp = phD.tile([RPC, D], F32, tag="outp")
                nc.vector.tensor_scalar_mul(outp[:, :], r2[:, :], ms2[:, :])
                nc.vector.tensor_mul(outp[:, :], outp[:, :], rms2b[:, :])
                nc.sync.dma_start(out=tt["t_out"].ap()[:, :], in_=outp[:, :])

    nc.finalize()
    return nc


def _prepare(inputs):
    """Host-side: fold weights, build per-core input maps."""
    f = {k: np.asarray(v, dtype=np.float32) for k, v in inputs.items()}
    s0, s1 = float(np.exp(f["log_scale"][0])), float(np.exp(f["log_scale"][1]))
    w1 = f["mlp_w1"]
    w1c0 = (w1 * s0).copy()
    w1c1 = (w1 * s1).copy()
    w1c0[0, :] /= EPSX
    w1c1[0, :] /= EPSX
    w1big = np.zeros((96, 384), np.float32)
    for c in range(3):
        w1big[32 * c:32 * c + 9, 128 * c:128 * c + 64] = w1c0
        w1big[32 * c + 9:32 * c + 18, 128 * c + 64:128 * c + 128] = w1c1

    sc18 = np.array([EPSX, 1, 1, 2, 2, 4, 4, 8, 8] * 2, np.float32)
    sb18 = np.array([0] + [0, PI / 2] * 4, np.float32)
    sb18 = np.concatenate([sb18, sb18])
    sinscale3 = np.ones((96, 1), np.float32)
    sinbias3 = np.zeros((96, 1), np.float32)
    for c in range(3):
        sinscale3[32 * c:32 * c + 18, 0] = sc18
        sinbias3[32 * c:32 * c + 18, 0] = sb18

    b1s = np.concatenate([f["mlp_b1"], f["mlp_b1"]]).reshape(128, 1)
    gam = np.concatenate([f["film_gamma"][0], f["film_gamma"][1]])  # (128,)
    bet = np.concatenate([f["film_beta"][0], f["film_beta"][1]])    # (128,)
    w2big = np.zeros((128, 128), np.float32)
    w2big[0:64, 0:64] = f["mlp_w2"]
    w2big[64:128, 64:128] = f["mlp_w2"]
    We = (w2big * gam[None, :]).astype(np.float32)      # e-hat = We^T h

    Wss_pat = We[:, 0:64] + We[:, 64:128]               # (128,64): s_c = e0_c+e1_c
    Wss2 = np.zeros((128, 256), np.float32)
    Wss2[:, 0:64] = Wss_pat
    Wss2[:, 192:256] = Wss_pat
    gw1 = f["gate_w1"]                                   # (128,64)
    Wgg_pat = (We @ gw1).astype(np.float32)
    Wgg2 = np.zeros((128, 256), np.float32)
    Wgg2[:, 0:64] = Wgg_pat
    Wgg2[:, 192:256] = Wgg_pat
    gb1p = f["gate_b1"] + gw1.T @ bet                    # (64,)
    gb1s = np.concatenate([gb1p, gb1p]).reshape(128, 1).astype(np.float32)
    bs64 = bet[0:64] + bet[64:128]
    bs128 = np.concatenate([bs64, bs64]).reshape(128, 1).astype(np.float32)
    has_bs = bool(np.any(bs64 != 0.0))

    temp = float(np.exp(f["gate_temp"]))
    gw2d = (f["gate_w2"][:, 0] - f["gate_w2"][:, 1]) / temp
    gb2d = float((f["gate_b2"][0] - f["gate_b2"][1]) / temp)
    gp = f["ln_g"] * f["proj_w"][:, 0]
    gps64 = float(gp.sum() / 64.0)

    ones64 = np.ones(64, np.float32)
    zer64 = np.zeros(64, np.float32)
    # stat slots: 0 a0, 1 a1, 2 b0, 3 b1, 4 q00, 5 q11, 6 qss, 7 d
    A_e = np.zeros((128, 8), np.float32)
    A_e[0:64, 0] = ones64 / 64.0
    A_e[64:128, 1] = ones64 / 64.0
    A_e[0:64, 2] = gp
    A_e[64:128, 3] = gp
    A_e[0:64, 4] = 2.0 * bet[0:64] / 64.0
    A_e[64:128, 5] = 2.0 * bet[64:128] / 64.0
    # stat-major stats tile: partition slot = s*16 + i_loc (i_loc = chunk%16).
    # sliding window: chunk i_loc uses cols [15-i_loc : 143-i_loc) of a
    # [128,143] tensor whose pattern sits at cols 15 + s*16.
    WstE = np.zeros((128, 143), np.float32)
    pat = We @ A_e                      # (128, 8)
    for s_ in range(8):
        WstE[:, 15 + 16 * s_] = pat[:, s_]
    WstQ = np.zeros((128, 143), np.float32)
    WstQ[0:64, 15 + 16 * 4] = ones64 / 64.0    # q00/64 from e-hat^2 (ch0)
    WstQ[64:128, 15 + 16 * 5] = ones64 / 64.0  # q11/64 (ch1)
    # pair windows slide by 2: pair p8 uses cols [14-2*p8 : 142-2*p8)
    WstS = np.zeros((128, 142), np.float32)
    WstS[0:64, 14 + 16 * 6] = ones64 / 64.0    # qss/64 even chunk
    WstS[64:128, 15 + 16 * 6] = ones64 / 64.0  # qss/64 odd chunk
    WstG = np.zeros((128, 142), np.float32)
    WstG[0:64, 14 + 16 * 7] = gw2d             # d even chunk
    WstG[64:128, 15 + 16 * 7] = gw2d           # d odd chunk

    ca0 = float(bet[0:64].sum()); ca1 = float(bet[64:128].sum())
    cb0 = float((gp * bet[0:64]).sum()); cb1 = float((gp * bet[64:128]).sum())
    cq00 = float((bet[0:64] ** 2).sum()); cq11 = float((bet[64:128] ** 2).sum())

    shared = {
        "w1big": w1big, "sinscale3": sinscale3, "sinbias3": sinbias3, "b1s": b1s,
        "We": We, "Wss2": Wss2, "Wgg2": Wgg2, "gb1s": gb1s, "bs128": bs128,
        "WstE": WstE, "WstQ": WstQ, "WstS": WstS, "WstG": WstG,
        "ident": np.eye(128, dtype=np.float32),
        "wq": f["wq"], "wk": f["wk"], "wv": f["wv"], "wo": f["wo"],
        "bq": f["bq"].reshape(128, 1),
        "bvb": np.broadcast_to(f["bv"], (128, H)).copy(),
        "bob": np.broadcast_to(f["bo"], (RPC, D)).copy(),
        "rms1b": np.broadcast_to(f["rms1"], (RPC, D)).copy(),
        "rms2b": np.broadcast_to(f["rms2"], (RPC, D)).copy(),
        "ffw1": f["ff_w1"],
        "ffb1s": f["ff_b1"].reshape(4, 128).T.copy(),
        "ffw2": np.concatenate([f["ff_w2"][128 * fb:128 * fb + 128, :] for fb in range(4)],
                               axis=1).copy(),
        "ffb2b": np.broadcast_to(f["ff_b2"], (RPC, D)).copy(),
    }

    in_maps = []
    for core in range(NCORES):
        b = core // 4
        r0 = (core % 4) * RPC
        xb = f["x"][b]
        cx = f["coords"][b, :, 0]
        cy = f["coords"][b, :, 1]
        m = dict(shared)
        m.update({
            "xT": np.ascontiguousarray(xb.T),
            "xrows": np.ascontiguousarray(xb[r0:r0 + RPC]),
            "xrowsT": np.ascontiguousarray(xb[r0:r0 + RPC].T),
            "cost_r": np.ascontiguousarray(f["cost_mat"][b, r0:r0 + RPC]),
            "clx": np.stack([cx[r0:r0 + RPC], np.ones(RPC, np.float32)]),
            "cly": np.stack([cy[r0:r0 + RPC], np.ones(RPC, np.float32)]),
            "crx": np.stack([np.ones(T, np.float32), -cx]),
            "cry": np.stack([np.ones(T, np.float32), -cy]),
        })
        in_maps.append(m)
    consts = {"gb2d": gb2d, "gps64": gps64, "has_bs": has_bs,
              "ca0": ca0, "ca1": ca1, "cb0": cb0, "cb1": cb1,
              "cq00": cq00, "cq11": cq11}
    return in_maps, consts


def _get_program(consts):
    key = tuple(sorted(consts.items()))
    if key not in _CACHE:
        nc, loc = _build_program()
        tt = {k: v for k, v in loc.items() if k.startswith("t_")}
        nc = _emit(nc, tt, consts)
        _CACHE[key] = nc
    return _CACHE[key]


def kernel(**inputs):
    in_maps, consts = _prepare(inputs)
    nc = _get_program(consts)
    res = bass_utils.run_bass_kernel_spmd(nc, in_maps, core_ids=list(range(NCORES)))
    out = np.zeros((B, T, D), np.float32)
    for core in range(NCORES):
        b = core // 4
        r0 = (core % 4) * RPC
        out[b, r0:r0 + RPC] = res.results[core]["out"]
    return out


# revision 39
# speedup vs baseline: 1.1058x; 1.1058x over previous
"""Trainium2 Bass kernel for nn_AttnFree_Block (AFT + neural adaptive pairwise bias).

Sharding: 8 cores over the (B=2, T=512) query-row grid -> 128 query rows/core.
Each core computes the full pairwise bias network for its 128 rows x 512 keys,
then the AFT attention + FFN for its rows. Output rows are gathered on host.

v1: software-pipelined bias-net loop. All consumers of e (gate hidden, channel
sums, stat projections) are refactored to read h via host-precomputed composite
weights (w2/gamma/beta folds), so the per-chunk cross-engine chain is just
h-mm -> silu -> 5 independent matmuls. Stats pack 16 chunks per PSUM tile.
fe rows are DMA broadcast-gathered instead of selector matmuls.
"""
import sys
sys.path.insert(0, '/opt/trn_rl_repo')

import math
import numpy as np

import concourse.bass as bass
import concourse.bacc as bacc
import concourse.mybir as mybir
import concourse.tile as tile
from concourse import bass_utils

F32 = mybir.dt.float32
F32R = mybir.dt.float32r
BF16 = mybir.dt.bfloat16

AF = mybir.ActivationFunctionType
ALU = mybir.AluOpType
AX = mybir.AxisListType

B, T, D, H, HID, FF = 2, 512, 128, 128, 64, 512
NCORES = 8
RPC = T * B // NCORES  # 128 query rows per core
FREQS = (1.0, 2.0, 4.0, 8.0)
EPS_LN = 1e-5
EPS_RMS = 1e-5
EPSX = 1e-4          # epsilon for the sin(eps*x)/eps ~= x identity-row trick
PI = math.pi

_CACHE = {}


def _build_program():
    nc = bacc.Bacc()

    def din(name, shape, dt=F32):
        return nc.dram_tensor(name, list(shape), dt, kind="ExternalInput")

    t_xT = din("xT", (D, T), F32R)
    t_xrows = din("xrows", (RPC, D))
    t_xrowsT = din("xrowsT", (D, RPC), F32R)
    t_cost = din("cost_r", (RPC, T))      # cost_mat rows for this core
    t_clx = din("clx", (2, RPC))
    t_cly = din("cly", (2, RPC))
    t_crx = din("crx", (2, T))
    t_cry = din("cry", (2, T))

    t_w1big = din("w1big", (96, 384), F32R)
    t_sinscale = din("sinscale3", (96, 1))
    t_sinbias = din("sinbias3", (96, 1))
    t_b1s = din("b1s", (128, 1))
    t_We = din("We", (128, 128), F32R)        # gamma-scaled blockdiag w2
    t_Wss2 = din("Wss2", (128, 256), F32R)    # even|odd pair sum-pattern
    t_Wgg2 = din("Wgg2", (128, 256), F32R)    # even|odd gate-hidden pattern
    t_gb1s = din("gb1s", (128, 1))
    t_bs128 = din("bs128", (128, 1))          # beta0+beta1 (sq-ss bias)
    t_WstE = din("WstE", (128, 143), F32R)    # sliding stat weights from h
    t_WstQ = din("WstQ", (128, 143), F32R)    # sliding stat weights from e^2
    t_WstS = din("WstS", (128, 142), F32R)    # sliding qss weights from ss^2
    t_WstG = din("WstG", (128, 142), F32R)    # sliding d weights from silu(gg)
    t_ident = din("ident", (128, 128))

    t_wq = din("wq", (D, H), F32R)
    t_wk = din("wk", (D, H), F32R)
    t_wv = din("wv", (D, H), F32R)
    t_wo = din("wo", (H, D), F32R)
    t_bq = din("bq", (H, 1))
    t_bvb = din("bvb", (128, H))
    t_bob = din("bob", (RPC, D))
    t_rms1b = din("rms1b", (RPC, D))
    t_rms2b = din("rms2b", (RPC, D))
    t_ffw1 = din("ffw1", (D, FF), F32R)
    t_ffb1s = din("ffb1s", (128, 4))
    t_ffw2 = din("ffw2", (D, FF), F32R)
    t_ffb2b = din("ffb2b", (RPC, D))

    t_angle_dram = nc.dram_tensor("angle_scratch", [RPC, T], F32, kind="Internal")
    t_out = nc.dram_tensor("out", [RPC, D], F32, kind="ExternalOutput")
    return nc, locals()


def _emit(nc, tt, consts):
    gb2d = consts["gb2d"]
    gps64 = consts["gps64"]
    NCH = RPC  # 128 chunks (query rows) per core

    with tile.TileContext(nc) as tc:
        import contextlib
        with contextlib.ExitStack() as ctx:
            singles = ctx.enter_context(tc.tile_pool(name="singles", bufs=1))
            sb = ctx.enter_context(tc.tile_pool(name="sb", bufs=1))

            def load1(t, shape, pool=None):
                p = pool or singles
                s = p.tile(list(shape), t.dtype, tag=t.name + "_sb", name=t.name + "_sb")
                nc.sync.dma_start(out=s[:, :], in_=t.ap()[:, :])
                return s

            # ---- load weights/statics (phase A + D-prep inputs first) ----
            clx = load1(tt["t_clx"], (2, RPC))
            cly = load1(tt["t_cly"], (2, RPC))
            crx = load1(tt["t_crx"], (2, T))
            cry = load1(tt["t_cry"], (2, T))
            xT = load1(tt["t_xT"], (D, T))
            xrowsT = load1(tt["t_xrowsT"], (D, RPC))
            wq = load1(tt["t_wq"], (D, H))
            wk = load1(tt["t_wk"], (D, H))
            wv = load1(tt["t_wv"], (D, H))
            ident = load1(tt["t_ident"], (128, 128))
            bq = load1(tt["t_bq"], (H, 1))
            bvb = load1(tt["t_bvb"], (128, H))
            w1big = load1(tt["t_w1big"], (96, 384))
            sinscale = load1(tt["t_sinscale"], (96, 1))
            sinbias = load1(tt["t_sinbias"], (96, 1))
            b1s = load1(tt["t_b1s"], (128, 1))
            We = load1(tt["t_We"], (128, 128))
            Wss2 = load1(tt["t_Wss2"], (128, 256))
            Wgg2 = load1(tt["t_Wgg2"], (128, 256))
            gb1s = load1(tt["t_gb1s"], (128, 1))
            WstE = load1(tt["t_WstE"], (128, 143))
            WstQ = load1(tt["t_WstQ"], (128, 143))
            WstS = load1(tt["t_WstS"], (128, 142))
            WstG = load1(tt["t_WstG"], (128, 142))
            bs128 = load1(tt["t_bs128"], (128, 1))

            def load_tail_weights():
                return (load1(tt["t_xrows"], (RPC, D)),
                        load1(tt["t_wo"], (H, D)),
                        load1(tt["t_bob"], (RPC, D)),
                        load1(tt["t_rms1b"], (RPC, D)),
                        load1(tt["t_rms2b"], (RPC, D)),
                        load1(tt["t_ffw1"], (D, FF)),
                        load1(tt["t_ffb1s"], (128, 4)),
                        load1(tt["t_ffw2"], (D, FF)),
                        load1(tt["t_ffb2b"], (RPC, D)))

            c_epsln = singles.tile([128, 1], F32, tag="c_epsln")
            nc.vector.memset(c_epsln[:, :], EPS_LN)
            c_epsrms = singles.tile([128, 1], F32, tag="c_epsrms")
            nc.vector.memset(c_epsrms[:, :], EPS_RMS)
            c_gb2d = singles.tile([128, 1], F32, tag="c_gb2d")
            nc.vector.memset(c_gb2d[:, :], gb2d)

            # ================= Phase A: angle matrix =================
            with tc.tile_pool(name="pA", bufs=2, space="PSUM") as pA:
                dx_ps = pA.tile([RPC, T], F32, tag="dxy")
                nc.tensor.matmul(dx_ps[:, :], clx[:, :], crx[:, :], start=True, stop=True)
                dy_ps = pA.tile([RPC, T], F32, tag="dxy")
                nc.tensor.matmul(dy_ps[:, :], cly[:, :], cry[:, :], start=True, stop=True)
                sgn = sb.tile([RPC, T], F32, tag="sgn")
                nc.scalar.sign(sgn[:, :], dx_ps[:, :])
                dxa = sb.tile([RPC, T], F32, tag="dxa")
                nc.vector.tensor_mul(dxa[:, :], sgn[:, :], dx_ps[:, :])
                cmpneg = sb.tile([RPC, T], F32, tag="cmpneg")
                nc.vector.tensor_scalar(cmpneg[:, :], sgn[:, :], 0.0, None, op0=ALU.is_lt)
                sdy = sb.tile([RPC, T], F32, tag="sdy")
                nc.vector.tensor_scalar(sdy[:, :], dy_ps[:, :], 0.0, None, op0=ALU.is_ge)
                nc.vector.tensor_scalar(sdy[:, :], sdy[:, :], 2.0, -1.0, op0=ALU.mult, op1=ALU.add)
                dya = sb.tile([RPC, T], F32, tag="dya")
                nc.vector.tensor_mul(dya[:, :], sdy[:, :], dy_ps[:, :])
                mx = sb.tile([RPC, T], F32, tag="mx")
                nc.vector.tensor_tensor(out=mx[:, :], in0=dxa[:, :], in1=dya[:, :], op=ALU.max)
                nc.vector.tensor_scalar_max(mx[:, :], mx[:, :], 1e-30)
                mn = sb.tile([RPC, T], F32, tag="mn")
                nc.vector.tensor_tensor(out=mn[:, :], in0=dxa[:, :], in1=dya[:, :], op=ALU.min)
                nc.vector.reciprocal(mx[:, :], mx[:, :])
                rt = sb.tile([RPC, T], F32, tag="rt")
                nc.vector.tensor_mul(rt[:, :], mn[:, :], mx[:, :])
                at = sb.tile([RPC, T], F32, tag="at")
                nc.scalar.activation(at[:, :], rt[:, :], AF.Arctan)
                swap = sb.tile([RPC, T], F32, tag="swap")
                nc.vector.tensor_tensor(out=swap[:, :], in0=dya[:, :], in1=dxa[:, :], op=ALU.is_gt)
                v1 = sb.tile([RPC, T], F32, tag="v1")
                nc.vector.tensor_scalar(v1[:, :], at[:, :], -2.0, PI / 2, op0=ALU.mult, op1=ALU.add)
                nc.vector.tensor_mul(v1[:, :], v1[:, :], swap[:, :])
                base = sb.tile([RPC, T], F32, tag="base")
                nc.vector.tensor_add(base[:, :], at[:, :], v1[:, :])
                v2 = sb.tile([RPC, T], F32, tag="v2")
                nc.vector.tensor_scalar(v2[:, :], base[:, :], -2.0, PI, op0=ALU.mult, op1=ALU.add)
                nc.vector.tensor_mul(v2[:, :], v2[:, :], cmpneg[:, :])
                nc.vector.tensor_add(base[:, :], base[:, :], v2[:, :])
                angle = sb.tile([RPC, T], F32, tag="angle")
                nc.vector.tensor_mul(angle[:, :], sdy[:, :], base[:, :])
                nc.sync.dma_start(out=tt["t_angle_dram"].ap()[:, :], in_=angle[:, :])

            # ============ Phase D-prep: Q_sig, K-softmax, eK, eKT, ekv ============
            qs_sb = singles.tile([H, RPC], F32, tag="qs_sb")
            eKT = singles.tile([128, T], F32R, tag="eKT")
            ekv = singles.tile([128, T], F32R, tag="ekv")
            with tc.tile_pool(name="dprep", bufs=1) as dprep, \
                 tc.tile_pool(name="psW", bufs=2, space="PSUM") as psW, \
                 tc.tile_pool(name="psK", bufs=1, space="PSUM") as psK:
                q_ps = psW.tile([H, RPC], F32, tag="q_ps")
                nc.tensor.matmul(q_ps[:, :], wq[:, :], xrowsT[:, :], start=True, stop=True)
                nc.scalar.activation(qs_sb[:, :], q_ps[:, :], AF.Sigmoid, bias=bq[:, :])

                kl_ps = psK.tile([H, T], F32, tag="kl_ps")
                nc.tensor.matmul(kl_ps[:, :], wk[:, :], xT[:, :], start=True, stop=True)
                kmax = dprep.tile([H, 1], F32, tag="kmax")
                nc.vector.reduce_max(kmax[:, :], kl_ps[:, :], axis=AX.X, negate=True)
                kex = dprep.tile([H, T], F32, tag="kex")
                nc.scalar.activation(kex[:, :], kl_ps[:, :], AF.Exp, bias=kmax[:, :])
                ksum = dprep.tile([H, 1], F32, tag="ksum")
                nc.vector.reduce_sum(ksum[:, :], kex[:, :], axis=AX.X)
                nc.vector.reciprocal(ksum[:, :], ksum[:, :])
                eK = dprep.tile([H, T], F32, tag="eK")
                nc.scalar.activation(eK[:, :], kex[:, :], AF.Exp, scale=ksum[:, :])
                for tb in range(4):
                    tp = psW.tile([128, 128], F32, tag="scrW")
                    nc.tensor.transpose(tp[:, :], eK[:, 128 * tb:128 * tb + 128], ident[:, :])
                    nc.vector.tensor_copy(out=eKT[:, 128 * tb:128 * tb + 128], in_=tp[:, :])
                for tb in range(4):
                    v_ps = psW.tile([128, H], F32, tag="scrW")
                    nc.tensor.matmul(v_ps[:, :], xT[:, 128 * tb:128 * tb + 128], wv[:, :],
                                     start=True, stop=True)
                    vb = dprep.tile([128, H], F32R, tag="vb")
                    nc.vector.tensor_add(vb[:, :], v_ps[:, :], bvb[:, :])
                    nc.vector.tensor_mul(ekv[:, 128 * tb:128 * tb + 128], vb[:, :],
                                         eKT[:, 128 * tb:128 * tb + 128])

            # ================= Phase B: pipelined bias-net main loop =================
            NG = (NCH + 2) // 3  # fe groups of 3 chunks (base partitions 0/32/64)
            Sb = [singles.tile([128, T], F32, tag=f"Sblk{b}", name=f"Sblk{b}")
                  for b in range(NCH // 16)]

            with tc.tile_pool(name="feRaw", bufs=2) as feRaw, \
                 tc.tile_pool(name="feSb", bufs=2) as feSb, \
                 tc.tile_pool(name="hSb", bufs=3) as hSb, \
                 tc.tile_pool(name="sqSb", bufs=2) as sqSb, \
                 tc.tile_pool(name="ssSb", bufs=2) as ssSb, \
                 tc.tile_pool(name="ggSb", bufs=2) as ggSb, \
                 tc.tile_pool(name="psHE", bufs=3, space="PSUM") as psHE, \
                 tc.tile_pool(name="psSG", bufs=3, space="PSUM") as psSG, \
                 tc.tile_pool(name="psST", bufs=2, space="PSUM") as psST:

                st = {"fe_raw": {}, "fe_sb": {}, "h_ps": {}, "h_sb": {}, "e_ps": {},
                      "sq_sb": {}, "ss_ps": {}, "gg_ps": {}, "ss_sb": {}, "gg_sb": {},
                      "stats": {}}
                prologue = True

                def dma_fe(g):
                    n = min(3, NCH - 3 * g)
                    fr = feRaw.tile([96, T], F32, tag="fe_raw")
                    st["fe_raw"][g] = (fr, n)
                    if g < 2:
                        nc.vector.memset(fr[:, :], 0.0)
                    for c in range(n):
                        i = 3 * g + c
                        src_c = bass.AP(tensor=tt["t_cost"], offset=i * T, ap=[[0, 9], [1, T]])
                        nc.sync.dma_start(out=fr[32 * c:32 * c + 9, :], in_=src_c)
                        src_a = bass.AP(tensor=tt["t_angle_dram"], offset=i * T, ap=[[0, 9], [1, T]])
                        nc.sync.dma_start(out=fr[32 * c + 9:32 * c + 18, :], in_=src_a)

                def sin_g(g):
                    fr, n = st["fe_raw"][g]
                    fs = feSb.tile([96, T], F32R, tag="fe_sb")
                    st["fe_sb"][g] = fs
                    nc.scalar.activation(fs[:, :], fr[:, :], AF.Sin,
                                         scale=sinscale[:, :], bias=sinbias[:, :])

                def h_mm(c):
                    g, c3 = c // 3, c % 3
                    fs = st["fe_sb"][g]
                    hp = psHE.tile([128, T], F32, tag="he_ps")
                    st["h_ps"][c] = hp
                    nc.tensor.matmul(hp[:, :], w1big[:, 128 * c3:128 * c3 + 128],
                                     fs[0:96, :], start=True, stop=True)

                def silu_h(c):
                    hp = st["h_ps"].pop(c)
                    hs = hSb.tile([128, T], F32R, tag="h_sb")
                    st["h_sb"][c] = hs
                    nc.scalar.activation(hs[:, :], hp[:, :], AF.Silu, bias=b1s[:, :])

                for t in range(-2, NCH + 3):
                    if prologue:
                        dma_fe(0)
                        dma_fe(1)
                        sin_g(0)
                        prologue = False
                    # vector: square of pair sums (reads ss PSUM once), frees ss_ps
                    if t % 2 == 0 and 0 <= (t - 2) // 2 < NCH // 2:
                        p = (t - 2) // 2
                        ssp = st["ss_ps"].pop(p)
                        tmp = ssSb.tile([128, T], F32, tag="ss_tmp")
                        nc.vector.tensor_scalar(tmp[:, :], ssp[:, :], bs128[:, :],
                                                None, op0=ALU.add)
                        sss = ssSb.tile([128, T], F32R, tag="ss_sb")
                        st["ss_sb"][p] = sss
                        nc.vector.tensor_mul(sss[:, :], tmp[:, :], tmp[:, :])
                    # scalar: silu of pair gate hidden (reads PSUM), frees gg_ps
                    if t % 2 == 0 and 0 <= (t - 2) // 2 < NCH // 2:
                        p = (t - 2) // 2
                        ggp = st["gg_ps"].pop(p)
                        ggs = ggSb.tile([128, T], F32R, tag="gg_sb")
                        st["gg_sb"][p] = ggs
                        nc.scalar.activation(ggs[:, :], ggp[:, :], AF.Silu, bias=gb1s[:, :])
                    # tensor: h two slots ahead; scalar: its silu right after
                    if 0 <= t + 2 < NCH:
                        h_mm(t + 2)
                        silu_h(t + 2)
                    # scalar: sin for group (t+3)//3 (3-slot lead)
                    if (t + 3) % 3 == 0 and 0 < (t + 3) // 3 < NG:
                        sin_g((t + 3) // 3)
                    # dma: fe gather for group t//3+2 (5-slot lead)
                    if t % 3 == 0 and 1 < t // 3 + 2 < NG:
                        dma_fe(t // 3 + 2)

                    if 0 <= t < NCH:
                        c = t
                        c16, hp2 = c % 16, c % 2
                        hs = st["h_sb"][c]
                        # e-hat matmul
                        ep = psHE.tile([128, T], F32, tag="he_ps")
                        st["e_ps"][c] = ep
                        nc.tensor.matmul(ep[:, :], We[:, :], hs[:, :], start=True, stop=True)
                        if hp2 == 0:
                            ssp = psSG.tile([128, T], F32, tag="sg_ps")
                            ggp = psSG.tile([128, T], F32, tag="sg_ps")
                            st["ss_ps"][c // 2] = ssp
                            st["gg_ps"][c // 2] = ggp
                        ssp = st["ss_ps"][c // 2]
                        nc.tensor.matmul(ssp[:, :], Wss2[:, 128 * hp2:128 * hp2 + 128],
                                         hs[:, :], start=(hp2 == 0), stop=(hp2 == 1))
                        # linear stats from h
                        if c16 == 0:
                            stp = psST.tile([128, T], F32, tag="st_ps")
                            st["stats"][c // 16] = stp
                        stp = st["stats"][c // 16]
                        nc.tensor.matmul(stp[:, :], WstE[:, 15 - c16:143 - c16],
                                         hs[:, :], start=(c16 == 0), stop=False)
                        # scalar squares e-hat straight from PSUM (no copy)
                        sqs = sqSb.tile([128, T], F32R, tag="sq_sb")
                        st["sq_sb"][c] = sqs
                        nc.scalar.activation(sqs[:, :], ep[:, :], AF.Square)

                    # tensor: quadratic stats of previous chunk
                    if 0 <= t - 1 < NCH:
                        c = t - 1
                        c16 = c % 16
                        sqs = st["sq_sb"].pop(c)
                        stp = st["stats"][c // 16]
                        nc.tensor.matmul(stp[:, :], WstQ[:, 15 - c16:143 - c16],
                                         sqs[:, :], start=False, stop=False)
                        st["e_ps"].pop(c, None)

                    # tensor: pair stats (qss, d)
                    if t % 2 == 1 and 0 <= (t - 3) // 2 < NCH // 2:
                        p = (t - 3) // 2
                        p8 = p % 8
                        sss = st["ss_sb"].pop(p)
                        ggs = st["gg_sb"].pop(p)
                        stp = st["stats"][p // 8]
                        nc.tensor.matmul(stp[:, :], WstS[:, 14 - 2 * p8:142 - 2 * p8],
                                         sss[:, :], start=False, stop=False)
                        nc.tensor.matmul(stp[:, :], WstG[:, 14 - 2 * p8:142 - 2 * p8],
                                         ggs[:, :], start=False, stop=(p8 == 7))

                    # tensor: gg accumulation last (bufs=3 rotation hazard dodge)
                    if 0 <= t < NCH:
                        c = t
                        hp2 = c % 2
                        hs = st["h_sb"][c]
                        ggp = st["gg_ps"][c // 2]
                        nc.tensor.matmul(ggp[:, :], Wgg2[:, 128 * hp2:128 * hp2 + 128],
                                         hs[:, :], start=(hp2 == 0), stop=(hp2 == 1))
                        st["h_sb"].pop(c, None)

                    # vector: completed stats block PSUM -> SBUF
                    if t >= 18 and (t - 18) % 16 == 0 and (t - 18) // 16 < NCH // 16:
                        b = (t - 18) // 16
                        stp = st["stats"].pop(b)
                        nc.vector.tensor_copy(out=Sb[b][:, :], in_=stp[:, :])

            (xrows, wo, bob, rms1b, rms2b, ffw1, ffb1s, ffw2, ffb2b) = load_tail_weights()

            # ================= Phase B2: transpose stats to key-major =================
            T_coll = singles.tile([128, 4096], F32, tag="Tcoll")
            TvS = T_coll[:, :].rearrange("p (s jb i) -> p jb s i", s=8, jb=4)
            with tc.tile_pool(name="psTr", bufs=2, space="PSUM") as psTr:
                for b in range(NCH // 16):
                    trp = psTr.tile([128, 512], F32, tag="trp")
                    for jb in range(4):
                        nc.tensor.transpose(trp[:, 128 * jb:128 * jb + 128],
                                            Sb[b][:, 128 * jb:128 * jb + 128], ident[:, :])
                    nc.vector.tensor_copy(
                        out=TvS[:, :, :, 16 * b:16 * b + 16],
                        in_=trp[:, :].rearrange("p (jb s i) -> p jb s i", jb=4, s=8))

            # ================= Phase C: assemble adaptive bias =================
            ab_sb = singles.tile([128, T], F32, tag="ab_sb")
            with tc.tile_pool(name="ph2", bufs=1) as ph2, \
                 tc.tile_pool(name="psC", bufs=2, space="PSUM") as psC:
                S = [T_coll[:, 512 * k:512 * k + 512] for k in range(8)]

                def dt(tag):
                    return ph2.tile([128, T], F32, tag=tag, name=tag)

                def cadd(ap, const):
                    if const != 0.0:
                        nc.vector.tensor_scalar(ap, ap, const, None, op0=ALU.add)

                g0t = dt("g0t")
                nc.scalar.activation(g0t[:, :], S[7], AF.Sigmoid, bias=c_gb2d[:, :])

                # Stat weights are pre-scaled host-side (a-slots 1/64, q-slots
                # 1/64) so the whole chain is tensor-tensor ops, splittable
                # across engines. s0 = mean, s2 = E[f^2], P = 2*q01/64.
                za = dt("za"); s0 = dt("s0"); zb = dt("zb"); s1 = dt("s1")
                t1 = dt("t1"); qp = dt("qp"); ss5 = dt("ss5"); zq2 = dt("zq2")
                spq = dt("spq"); zv = dt("zv"); v1 = dt("v1"); v2 = dt("v2")
                v3 = dt("v3"); s2 = dt("s2"); m2 = dt("m2")
                var = dt("var")
                rstd = dt("rstd")
                num1 = dt("num1"); abT = dt("abT")
                SPL = 320

                def half(eng, a, b):
                    cut = lambda ap: ap[:, a:b]
                    Sh = [S[k][:, a:b] for k in range(8)]

                    def TT(o, i0, i1, op):
                        eng.tensor_tensor(out=o, in0=i0, in1=i1, op=op)

                    def cadd(ap, const):
                        if const != 0.0:
                            nc.vector.tensor_scalar(ap, ap, const, None, op0=ALU.add)

                    TT(cut(za[:, :]), Sh[0], Sh[1], ALU.subtract)
                    cadd(cut(za[:, :]), (consts["ca0"] - consts["ca1"]) / 64.0)
                    TT(cut(s0[:, :]), cut(g0t[:, :]), cut(za[:, :]), ALU.mult)
                    TT(cut(s0[:, :]), cut(s0[:, :]), Sh[1], ALU.add)
                    cadd(cut(s0[:, :]), consts["ca1"] / 64.0)
                    TT(cut(zb[:, :]), Sh[2], Sh[3], ALU.subtract)
                    cadd(cut(zb[:, :]), consts["cb0"] - consts["cb1"])
                    TT(cut(s1[:, :]), cut(g0t[:, :]), cut(zb[:, :]), ALU.mult)
                    TT(cut(s1[:, :]), cut(s1[:, :]), Sh[3], ALU.add)
                    cadd(cut(s1[:, :]), consts["cb1"])
                    # P = 2*q01/64 = S6 - S4 - S5 (all 1/64-scaled)
                    TT(cut(t1[:, :]), Sh[6], Sh[4], ALU.subtract)
                    TT(cut(qp[:, :]), cut(t1[:, :]), Sh[5], ALU.subtract)
                    cadd(cut(qp[:, :]), (-consts["cq00"] - consts["cq11"]) / 64.0)
                    TT(cut(ss5[:, :]), Sh[5], Sh[5], ALU.add)
                    TT(cut(zq2[:, :]), cut(qp[:, :]), cut(ss5[:, :]), ALU.subtract)
                    cadd(cut(zq2[:, :]), -2.0 * consts["cq11"] / 64.0)
                    TT(cut(spq[:, :]), Sh[4], Sh[5], ALU.add)
                    cadd(cut(spq[:, :]), (consts["cq00"] + consts["cq11"]) / 64.0)
                    TT(cut(zv[:, :]), cut(spq[:, :]), cut(qp[:, :]), ALU.subtract)
                    TT(cut(v1[:, :]), cut(g0t[:, :]), cut(zv[:, :]), ALU.mult)
                    TT(cut(v2[:, :]), cut(zq2[:, :]), cut(v1[:, :]), ALU.add)
                    TT(cut(v3[:, :]), cut(g0t[:, :]), cut(v2[:, :]), ALU.mult)
                    TT(cut(s2[:, :]), cut(v3[:, :]), Sh[5], ALU.add)
                    cadd(cut(s2[:, :]), consts["cq11"] / 64.0)
                    TT(cut(m2[:, :]), cut(s0[:, :]), cut(s0[:, :]), ALU.mult)
                    TT(cut(var[:, :]), cut(s2[:, :]), cut(m2[:, :]), ALU.subtract)

                half(nc.vector, 0, SPL)
                half(nc.gpsimd, SPL, T)
                sdv = dt("sdv")
                nc.scalar.activation(sdv[:, :], var[:, :], AF.Sqrt, bias=c_epsln[:, :])
                nc.vector.reciprocal(rstd[:, :], sdv[:, :])
                gpsum = consts["gps64"] * 64.0
                nc.vector.scalar_tensor_tensor(num1[:, :], s0[:, :], -gpsum, s1[:, :],
                                               op0=ALU.mult, op1=ALU.add)
                nc.vector.tensor_mul(abT[:, 0:SPL], num1[:, 0:SPL], rstd[:, 0:SPL])
                nc.gpsimd.tensor_mul(abT[:, SPL:T], num1[:, SPL:T], rstd[:, SPL:T])
                for jb in range(4):
                    tp = psC.tile([128, 128], F32, tag="tpC")
                    nc.tensor.transpose(tp[:, :], abT[:, 128 * jb:128 * jb + 128], ident[:, :])
                    nc.vector.tensor_copy(out=ab_sb[:, 128 * jb:128 * jb + 128], in_=tp[:, :])

            # ================= Phase D: AFT attention + FFN =================
            with tc.tile_pool(name="phD", bufs=1) as phD, \
                 tc.tile_pool(name="psD", bufs=3, space="PSUM") as psD, \
                 tc.tile_pool(name="psD2", bufs=1, space="PSUM") as psD2:
                rmax = phD.tile([128, 1], F32, tag="rmax")
                nc.vector.reduce_max(rmax[:, :], ab_sb[:, :], axis=AX.X, negate=True)
                ex = phD.tile([128, T], F32, tag="ex")
                nc.scalar.activation(ex[:, :], ab_sb[:, :], AF.Exp, bias=rmax[:, :])
                rsum = phD.tile([128, 1], F32, tag="rsum")
                nc.vector.reduce_sum(rsum[:, :], ex[:, :], axis=AX.X)
                nc.vector.reciprocal(rsum[:, :], rsum[:, :])
                A_sb = phD.tile([128, T], F32, tag="A_sb")
                nc.scalar.activation(A_sb[:, :], ex[:, :], AF.Exp, scale=rsum[:, :])
                AT_sb = phD.tile([128, T], F32R, tag="AT_sb")
                for jb in range(4):
                    tp = psD.tile([128, 128], F32, tag="scrD")
                    nc.tensor.transpose(tp[:, :], A_sb[:, 128 * jb:128 * jb + 128], ident[:, :])
                    nc.vector.tensor_copy(out=AT_sb[:, 128 * jb:128 * jb + 128], in_=tp[:, :])

                num_ps = psD2.tile([H, RPC], F32, tag="num_ps")
                den_ps = psD2.tile([H, RPC], F32, tag="den_ps")
                for jb in range(4):
                    nc.tensor.matmul(num_ps[:, :], ekv[:, 128 * jb:128 * jb + 128],
                                     AT_sb[:, 128 * jb:128 * jb + 128],
                                     start=(jb == 0), stop=(jb == 3))
                for jb in range(4):
                    nc.tensor.matmul(den_ps[:, :], eKT[:, 128 * jb:128 * jb + 128],
                                     AT_sb[:, 128 * jb:128 * jb + 128],
                                     start=(jb == 0), stop=(jb == 3))
                rden = phD.tile([H, RPC], F32, tag="rden")
                nc.vector.reciprocal(rden[:, :], den_ps[:, :])
                attT = phD.tile([H, RPC], F32R, tag="attT")
                nc.vector.tensor_mul(attT[:, :], num_ps[:, :], rden[:, :])
                nc.vector.tensor_mul(attT[:, :], attT[:, :], qs_sb[:, :])

                att_ps = psD2.tile([RPC, D], F32, tag="att_ps")
                nc.tensor.matmul(att_ps[:, :], attT[:, :], wo[:, :], start=True, stop=True)
                r1 = phD.tile([RPC, D], F32, tag="r1")
                nc.vector.tensor_add(r1[:, :], att_ps[:, :], xrows[:, :])
                nc.vector.tensor_add(r1[:, :], r1[:, :], bob[:, :])

                sq1 = phD.tile([RPC, D], F32, tag="sq1")
                nc.gpsimd.tensor_mul(sq1[:, :], r1[:, :], r1[:, :])
                ms = phD.tile([RPC, 1], F32, tag="ms")
                nc.vector.reduce_sum(ms[:, :], sq1[:, :], axis=AX.X)
                nc.scalar.activation(ms[:, :], ms[:, :], AF.Sqrt, scale=1.0 / D, bias=c_epsrms[0:RPC, :])
                nc.vector.reciprocal(ms[:, :], ms[:, :])
                h1 = phD.tile([RPC, D], F32, tag="h1")
                nc.vector.tensor_scalar_mul(h1[:, :], r1[:, :], ms[:, :])
                nc.vector.tensor_mul(h1[:, :], h1[:, :], rms1b[:, :])

                h1T_ps = psD.tile([D, RPC], F32, tag="scrD")
                nc.tensor.transpose(h1T_ps[:, :], h1[:, :], ident[:, :])
                h1T = phD.tile([D, RPC], F32R, tag="h1T")
                nc.vector.tensor_copy(out=h1T[:, :], in_=h1T_ps[:, :])

                relu_sb = phD.tile([128, FF], F32R, tag="relu_sb")
                for fb in range(4):
                    f_ps = psD.tile([128, RPC], F32, tag="scrD")
                    nc.tensor.matmul(f_ps[:, :], ffw1[:, 128 * fb:128 * fb + 128], h1T[:, :],
                                     start=True, stop=True)
                    nc.scalar.activation(relu_sb[:, 128 * fb:128 * fb + 128], f_ps[:, :],
                                         AF.Relu, bias=ffb1s[:, fb:fb + 1])
                o2_ps = psD2.tile([RPC, D], F32, tag="o2_ps")
                for fb in range(4):
                    nc.tensor.matmul(o2_ps[:, :], relu_sb[:, 128 * fb:128 * fb + 128],
                                     ffw2[:, 128 * fb:128 * fb + 128],
                                     start=(fb == 0), stop=(fb == 3))
                r2 = phD.tile([RPC, D], F32, tag="r2")
                nc.vector.tensor_add(r2[:, :], o2_ps[:, :], h1[:, :])
                nc.vector.tensor_add(r2[:, :], r2[:, :], ffb2b[:, :])

                sq2 = phD.tile([RPC, D], F32, tag="sq2")
                nc.gpsimd.tensor_mul(sq2[:, :], r2[:, :], r2[:, :])
                ms2 = phD.tile([RPC, 1], F32, tag="ms2")
                nc.vector.reduce_sum(ms2[:, :], sq2[:, :], axis=AX.X)
                nc.scalar.activation(ms2[:, :], ms2[:, :], AF.Sqrt, scale=1.0 / D, bias=c_epsrms[0:RPC, :])
                nc.vector.reciprocal(ms2[:, :], ms2[:, :])
                outp = phD.tile([RPC, D], F32, tag="outp")
                nc.vector.tensor_scalar_mul(outp[:, :], r2[:, :], ms2[:, :])
                nc.vector.tensor_mul(outp[:, :], outp[:, :], rms2b[:, :])
                nc.sync.dma_start(out=tt["t_out"].ap()[:, :], in_=outp[:, :])

    nc.finalize()
    return nc


def _prepare(inputs):
    """Host-side: fold weights, build per-core input maps."""
    f = {k: np.asarray(v, dtype=np.float32) for k, v in inputs.items()}
    s0, s1 = float(np.exp(f["log_scale"][0])), float(np.exp(f["log_scale"][1]))
    w1 = f["mlp_w1"]
    w1c0 = (w1 * s0).copy()
    w1c1 = (w1 * s1).copy()
    w1c0[0, :] /= EPSX
    w1c1[0, :] /= EPSX
    w1big = np.zeros((96, 384), np.float32)
    for c in range(3):
        w1big[32 * c:32 * c + 9, 128 * c:128 * c + 64] = w1c0
        w1big[32 * c + 9:32 * c + 18, 128 * c + 64:128 * c + 128] = w1c1

    sc18 = np.array([EPSX, 1, 1, 2, 2, 4, 4, 8, 8] * 2, np.float32)
    sb18 = np.array([0] + [0, PI / 2] * 4, np.float32)
    sb18 = np.concatenate([sb18, sb18])
    sinscale3 = np.ones((96, 1), np.float32)
    sinbias3 = np.zeros((96, 1), np.float32)
    for c in range(3):
        sinscale3[32 * c:32 * c + 18, 0] = sc18
        sinbias3[32 * c:32 * c + 18, 0] = sb18

    b1s = np.concatenate([f["mlp_b1"], f["mlp_b1"]]).reshape(128, 1)
    gam = np.concatenate([f["film_gamma"][0], f["film_gamma"][1]])  # (128,)
    bet = np.concatenate([f["film_beta"][0], f["film_beta"][1]])    # (128,)
    w2big = np.zeros((128, 128), np.float32)
    w2big[0:64, 0:64] = f["mlp_w2"]
    w2big[64:128, 64:128] = f["mlp_w2"]
    We = (w2big * gam[None, :]).astype(np.float32)      # e-hat = We^T h

    Wss_pat = We[:, 0:64] + We[:, 64:128]               # (128,64): s_c = e0_c+e1_c
    Wss2 = np.zeros((128, 256), np.float32)
    Wss2[:, 0:64] = Wss_pat
    Wss2[:, 192:256] = Wss_pat
    gw1 = f["gate_w1"]                                   # (128,64)
    Wgg_pat = (We @ gw1).astype(np.float32)
    Wgg2 = np.zeros((128, 256), np.float32)
    Wgg2[:, 0:64] = Wgg_pat
    Wgg2[:, 192:256] = Wgg_pat
    gb1p = f["gate_b1"] + gw1.T @ bet                    # (64,)
    gb1s = np.concatenate([gb1p, gb1p]).reshape(128, 1).astype(np.float32)
    bs64 = bet[0:64] + bet[64:128]
    bs128 = np.concatenate([bs64, bs64]).reshape(128, 1).astype(np.float32)
    has_bs = bool(np.any(bs64 != 0.0))

    temp = float(np.exp(f["gate_temp"]))
    gw2d = (f["gate_w2"][:, 0] - f["gate_w2"][:, 1]) / temp
    gb2d = float((f["gate_b2"][0] - f["gate_b2"][1]) / temp)
    gp = f["ln_g"] * f["proj_w"][:, 0]
    gps64 = float(gp.sum() / 64.0)

    ones64 = np.ones(64, np.float32)
    zer64 = np.zeros(64, np.float32)
    # stat slots: 0 a0, 1 a1, 2 b0, 3 b1, 4 q00, 5 q11, 6 qss, 7 d
    A_e = np.zeros((128, 8), np.float32)
    A_e[0:64, 0] = ones64 / 64.0
    A_e[64:128, 1] = ones64 / 64.0
    A_e[0:64, 2] = gp
    A_e[64:128, 3] = gp
    A_e[0:64, 4] = 2.0 * bet[0:64] / 64.0
    A_e[64:128, 5] = 2.0 * bet[64:128] / 64.0
    # stat-major stats tile: partition slot = s*16 + i_loc (i_loc = chunk%16).
    # sliding window: chunk i_loc uses cols [15-i_loc : 143-i_loc) of a
    # [128,143] tensor whose pattern sits at cols 15 + s*16.
    WstE = np.zeros((128, 143), np.float32)
    pat = We @ A_e                      # (128, 8)
    for s_ in range(8):
        WstE[:, 15 + 16 * s_] = pat[:, s_]
    WstQ = np.zeros((128, 143), np.float32)
    WstQ[0:64, 15 + 16 * 4] = ones64 / 64.0    # q00/64 from e-hat^2 (ch0)
    WstQ[64:128, 15 + 16 * 5] = ones64 / 64.0  # q11/64 (ch1)
    # pair windows slide by 2: pair p8 uses cols [14-2*p8 : 142-2*p8)
    WstS = np.zeros((128, 142), np.float32)
    WstS[0:64, 14 + 16 * 6] = ones64 / 64.0    # qss/64 even chunk
    WstS[64:128, 15 + 16 * 6] = ones64 / 64.0  # qss/64 odd chunk
    WstG = np.zeros((128, 142), np.float32)
    WstG[0:64, 14 + 16 * 7] = gw2d             # d even chunk
    WstG[64:128, 15 + 16 * 7] = gw2d           # d odd chunk

    ca0 = float(bet[0:64].sum()); ca1 = float(bet[64:128].sum())
    cb0 = float((gp * bet[0:64]).sum()); cb1 = float((gp * bet[64:128]).sum())
    cq00 = float((bet[0:64] ** 2).sum()); cq11 = float((bet[64:128] ** 2).sum())

    shared = {
        "w1big": w1big, "sinscale3": sinscale3, "sinbias3": sinbias3, "b1s": b1s,
        "We": We, "Wss2": Wss2, "Wgg2": Wgg2, "gb1s": gb1s, "bs128": bs128,
        "WstE": WstE, "WstQ": WstQ, "WstS": WstS, "WstG": WstG,
        "ident": np.eye(128, dtype=np.float32),
        "wq": f["wq"], "wk": f["wk"], "wv": f["wv"], "wo": f["wo"],
        "bq": f["bq"].reshape(128, 1),
        "bvb": np.broadcast_to(f["bv"], (128, H)).copy(),
        "bob": np.broadcast_to(f["bo"], (RPC, D)).copy(),
        "rms1b": np.broadcast_to(f["rms1"], (RPC, D)).copy(),
        "rms2b": np.broadcast_to(f["rms2"], (RPC, D)).copy(),
        "ffw1": f["ff_w1"],
        "ffb1s": f["ff_b1"].reshape(4, 128).T.copy(),
        "ffw2": np.concatenate([f["ff_w2"][128 * fb:128 * fb + 128, :] for fb in range(4)],
                               axis=1).copy(),
        "ffb2b": np.broadcast_to(f["ff_b2"], (RPC, D)).copy(),
    }

    in_maps = []
    for core in range(NCORES):
        b = core // 4
        r0 = (core % 4) * RPC
        xb = f["x"][b]
        cx = f["coords"][b, :, 0]
        cy = f["coords"][b, :, 1]
        m = dict(shared)
        m.update({
            "xT": np.ascontiguousarray(xb.T),
            "xrows": np.ascontiguousarray(xb[r0:r0 + RPC]),
            "xrowsT": np.ascontiguousarray(xb[r0:r0 + RPC].T),
            "cost_r": np.ascontiguousarray(f["cost_mat"][b, r0:r0 + RPC]),
            "clx": np.stack([cx[r0:r0 + RPC], np.ones(RPC, np.float32)]),
            "cly": np.stack([cy[r0:r0 + RPC], np.ones(RPC, np.float32)]),
            "crx": np.stack([np.ones(T, np.float32), -cx]),
            "cry": np.stack([np.ones(T, np.float32), -cy]),
        })
        in_maps.append(m)
    consts = {"gb2d": gb2d, "gps64": gps64, "has_bs": has_bs,
              "ca0": ca0, "ca1": ca1, "cb0": cb0, "cb1": cb1,
              "cq00": cq00, "cq11": cq11}
    return in_maps, consts


def _get_program(consts):
    key = tuple(sorted(consts.items()))
    if key not in _CACHE:
        nc, loc = _build_program()
        tt = {k: v for k, v in loc.items() if k.startswith("t_")}
        nc = _emit(nc, tt, consts)
        _CACHE[key] = nc
    return _CACHE[key]


def kernel(**inputs):
    in_maps, consts = _prepare(inputs)
    nc = _get_program(consts)
    res = bass_utils.run_bass_kernel_spmd(nc, in_maps, core_ids=list(range(NCORES)))
    out = np.zeros((B, T, D), np.float32)
    for core in range(NCORES):
        b = core // 4
        r0 = (core % 4) * RPC
        out[b, r0:r0 + RPC] = res.results[core]["out"]
    return out
